# revision 14
# baseline (speedup 1.0000x reference)
"""Trainium2 Bass kernel for nn_DepthwiseConvOverTimeLayer.

Pipeline (per core, C-sharded 8 ways):
  stage A: depthwise 3x3 conv as per-channel banded matmul on PE,
           fp8e4m3 DoubleRow (K=49 split 25+24+pad), bias deferred.
           PSUM groups of 2 channels x (A:ho<4 112 rows | B:ho>=4 84 rows).
  drain:   temporal max over t=20 out of fp32 PSUM, split across
           DVE (pairwise max / direct reduce), ACT (copy to fp16 SBUF),
           Pool/GPSIMD (fp16 max cascades). Path per 4-group block is
           tunable via PATHS.
  stage B: PE transpose per b -> ymT [ch, b, m, 9x9-padded halo].
  stage C: conv2 as 36 block-diag matmuls (fp16, 9-tap PSUM accum),
           bias (dw_b + conv_b folded into host map zb) added on exit.

Channel coupling: conv2 group c2 consumes depthwise channel
c = 256*m2 + c2//4 at multiplier m = c2 % 4; core j owns c2 in
[128j, 128j+128) -> dw channels {256*(l//32) + 32j + (l%32)}.
"""

import numpy as np
import ml_dtypes

B, T, H, W, C, M = 16, 20, 7, 7, 1024, 4
KD = KP = 3
NCORES = 8
CL = 128          # dw channels per core
HWQ = 49
BT = B * T        # 320
F16 = np.float16
F8 = ml_dtypes.float8_e4m3fn

# Per-group (1 channel each) drain path stripe (period 16). Even positions
# use DVE for the PSUM-exit, odd positions ACT, so the two PSUM-parity
# dependency chains each alternate engines and overlap:
# U1:  DVE pairwise max (t20->t10 into p10 ring) + Pool cascade lvl2.
# U3h: ACT copy (t20 into s2 ring) + DVE batched halve -> p10 + Pool cascade.
# U3p: ACT copy + Pool t20 halve + Pool cascade.
# U4:  DVE pairwise max + DVE batched reduce lvl2 (no Pool).
STRIPE16 = ["U1", "U3h", "U4", "U3p", "U1", "U3h", "U4", "U3p",
            "U1", "U3h", "U1", "U3h", "U4", "U3p", "U1", "U3h"]
_BASES = {"U1": 0, "U3h": 40, "U3p": 80, "U4": 104}


def _assignment():
    """Per-group (path, index-within-path); PERM[ymax col] = channel."""
    cnt = {p: 0 for p in _BASES}
    perm = np.zeros(CL, np.int64)
    plan = []
    for g in range(CL):
        path = STRIPE16[g % 16]
        i = cnt[path]
        cnt[path] += 1
        b, k = divmod(i, 8)
        perm[_BASES[path] + 8 * b + k] = g
        plan.append((path, i))
    return plan, perm


PLAN, PERM = _assignment()

TRACE = False
LAST_RESULTS = None
_NC_CACHE = None


# ----------------------------------------------------------------- host prep
def _core_channels(j):
    l = np.arange(CL)
    return 256 * (l // 32) + 32 * j + (l % 32)


def _col_of(m, ho, wo):
    """Stage-A output row index: A-half (ho<4) then B-half, m-major."""
    if ho < 4:
        return m * 28 + ho * 7 + wo
    return 112 + m * 21 + (ho - 4) * 7 + wo


def build_core_inputs(x, dw_w, dw_b, conv_w, conv_b, j):
    cj = _core_channels(j)

    # xb [25, 2, CL, BT] fp8: row hw_in = k*25+p, col bt = b*20+t
    xs = np.asarray(x, np.float32)[:, :, :, :, cj]           # [B,T,H,W,CL]
    xf = xs.transpose(2, 3, 4, 0, 1).reshape(HWQ, CL, BT)
    xb = np.zeros((50, CL, BT), np.float32)
    xb[:HWQ] = xf
    xb = xb.reshape(2, 25, CL, BT).transpose(1, 0, 2, 3)

    # aw [25, 2, CL, 196] fp8 banded depthwise lhsT (no bias row)
    wsel = np.asarray(dw_w, np.float32)[:, :, cj, :]         # [3,3,CL,4]
    aw = np.zeros((50, CL, 196), np.float32)
    for ho in range(H):
        for wo in range(W):
            for kh in range(KD):
                for kw in range(KD):
                    hi, wi = ho + kh - 1, wo + kw - 1
                    if 0 <= hi < H and 0 <= wi < W:
                        for m in range(M):
                            aw[hi * 7 + wi, :, _col_of(m, ho, wo)] = \
                                wsel[kh, kw, :, m]
    aw = aw.reshape(2, 25, CL, 196).transpose(1, 0, 2, 3)

    # a2 [CL, 36, CL] f16: conv2 block-diag weights.
    # a2[p_in=32*m2+s, t9*4+r, p_out=4*s+mo] = conv_w[c2=128j+4s+r, kh, kw, m2, mo]
    a2 = np.zeros((CL, 36, CL), np.float32)
    cw = np.asarray(conv_w, np.float32)
    s = np.arange(32)
    for t9 in range(9):
        kh, kw = divmod(t9, 3)
        for r in range(4):
            blk = cw[128 * j + 4 * s + r, kh, kw, :, :]       # [32, m2, mo]
            for m2 in range(4):
                a2[32 * m2 + s[:, None], t9 * 4 + r,
                   4 * s[:, None] + np.arange(4)[None, :]] = blk[:, m2, :]

    a2 = a2[PERM]

    # zb [CL, 4, 8, 49] f16: (dw_b + conv_b) folded through conv2, border-aware
    dwb_g = np.asarray(dw_b, np.float32).reshape(M, C)        # [m2, c2]
    c2s = 128 * j + np.arange(CL)
    # contrib[c2l, kh, kw, mo] = sum_m2 cw[c2, kh, kw, m2, mo] * dwb_g[m2, c2]
    contrib = np.einsum("ckhmn,mc->ckhn", cw[c2s], dwb_g[:, c2s])
    zmap = np.zeros((CL, M, HWQ), np.float32)                 # [c2l, mo, hw]
    cb = np.asarray(conv_b, np.float32)
    for ho in range(H):
        for wo in range(W):
            acc = cb[c2s].copy()                              # [c2l, mo]
            for kh in range(KD):
                for kw in range(KD):
                    if 0 <= ho + kh - 1 < H and 0 <= wo + kw - 1 < W:
                        acc += contrib[:, kh, kw]
            zmap[:, :, ho * 7 + wo] = acc
    # zb[po=4s+mo, r, hw] = zmap[c2l = 4s+r, mo, hw]
    zb = np.empty((CL, 4, HWQ), np.float32)
    for mo in range(4):
        for r in range(4):
            zb[4 * s + mo, r] = zmap[4 * s + r, mo]
    zb = np.repeat(zb[:, :, None, :], 8, axis=2)              # [CL, 4, 8, 49]

    ident = np.eye(112, dtype=np.float32)

    return {"xb": xb.astype(F8), "aw": aw.astype(F8),
            "a2": a2.astype(F16), "zb": zb.astype(F16),
            "ident": ident.astype(F16)}


def assemble_output(core_outs):
    """core_outs[j]['zout'] [CL=(4s+mo), 4=r, 16=b, 49=hw] -> (B,M,H,W,C)."""
    out = np.empty((B, M, H, W, C), np.float32)
    for j in range(NCORES):
        z = np.asarray(core_outs[j]["zout"]).astype(np.float32)
        z = z.reshape(32, 4, 4, B, HWQ)                 # s, mo, r, b, hw
        zz = z.transpose(3, 1, 4, 0, 2).reshape(B, M, H, W, CL)
        out[:, :, :, :, 128 * j:128 * j + 128] = zz
    return out


# ----------------------------------------------------------------- bass build
def build_bass():
    import concourse.mybir as mybir
    from concourse import bacc
    from concourse.tile import TileContext

    dt = mybir.dt
    op = mybir.AluOpType
    DR = mybir.MatmulPerfMode.DoubleRow
    nc = bacc.Bacc()

    xb_d = nc.dram_tensor("xb", [25, 2, CL, BT], dt.float8e4,
                          kind="ExternalInput")
    aw_d = nc.dram_tensor("aw", [25, 2, CL, 196], dt.float8e4,
                          kind="ExternalInput")
    a2_d = nc.dram_tensor("a2", [CL, 36, CL], dt.float16, kind="ExternalInput")
    zb_d = nc.dram_tensor("zb", [CL, 4, 8, HWQ], dt.float16,
                          kind="ExternalInput")
    ident_d = nc.dram_tensor("ident", [112, 112], dt.float16,
                             kind="ExternalInput")
    zout_d = nc.dram_tensor("zout", [CL, 4, B, HWQ], dt.float16,
                            kind="ExternalOutput")

    def stt_max(eng, out, in0, in1):
        eng.scalar_tensor_tensor(out, in0, 1.0, in1, op0=op.mult, op1=op.max)

    with TileContext(nc) as tc:
        with tc.tile_pool(name="const", bufs=1) as cpool:
            xb_t = cpool.tile([25, 2, CL, BT], dt.float8e4)
            aw_t = cpool.tile([25, 2, CL, 196], dt.float8e4)
            a2_t = cpool.tile([CL, 36, CL], dt.float16)
            zb_t = cpool.tile([CL, 4, 8, HWQ], dt.float16)
            ident_t = cpool.tile([112, 112], dt.float16)
            ymax = cpool.tile([112, CL, 2, B], dt.float16)  # [row, ch, A|B, b]
            ymT = cpool.tile([CL, B, 4, 81], dt.float16)    # [ch, b, m, 9x9]
            zsb = cpool.tile([CL, 4, 8, HWQ], dt.float16)
            p10 = cpool.tile([112, 3, 16, B, 10], dt.float16)  # pool ring
            s2d = cpool.tile([112, 2, 16, B, T], dt.float16)   # ACT copy ring
            r4d = cpool.tile([112, 16, B, 10], dt.float16)     # U4 DVE ring
            scd = cpool.tile([112, 16, B, 8], dt.float16)      # pool scratch

            nc.sync.dma_start(out=xb_t[:, :, 0:32, :], in_=xb_d[:, :, 0:32, :])
            nc.sync.dma_start(out=aw_t[:, :, 0:64, :], in_=aw_d[:, :, 0:64, :])
            nc.sync.dma_start(out=xb_t[:, :, 32:64, :],
                              in_=xb_d[:, :, 32:64, :])
            nc.sync.dma_start(out=xb_t[:, :, 64:96, :],
                              in_=xb_d[:, :, 64:96, :])
            nc.sync.dma_start(out=aw_t[:, :, 64:128, :],
                              in_=aw_d[:, :, 64:128, :])
            nc.sync.dma_start(out=xb_t[:, :, 96:128, :],
                              in_=xb_d[:, :, 96:128, :])
            nc.sync.dma_start(out=a2_t[:], in_=a2_d[:])
            nc.sync.dma_start(out=zb_t[:], in_=zb_d[:])
            nc.sync.dma_start(out=ident_t[:], in_=ident_d[:])

            # halo borders of ymT (interior is fully written in stage B)
            ymg = ymT.rearrange("p b m (hh ww) -> p b m hh ww", hh=9, ww=9)
            nc.gpsimd.memset(ymg[:, :, :, 0, :], 0.0)
            nc.gpsimd.memset(ymg[:, :, :, 8, :], 0.0)
            nc.gpsimd.memset(ymg[:, :, :, 1:8, 0], 0.0)
            nc.gpsimd.memset(ymg[:, :, :, 1:8, 8], 0.0)

            def pool_cascade(w, dest):
                """w [112,16,B,10] f16 -> max over t10 -> dest [112,8,2,B]."""
                sc = scd
                stt_max(nc.gpsimd, sc[:, :, :, 0:5], w[:, :, :, 0:5],
                        w[:, :, :, 5:10])
                stt_max(nc.gpsimd, sc[:, :, :, 5:7], sc[:, :, :, 0:2],
                        sc[:, :, :, 2:4])
                stt_max(nc.gpsimd, sc[:, :, :, 7:8], sc[:, :, :, 5:6],
                        sc[:, :, :, 6:7])
                stt_max(nc.gpsimd, dest,
                        sc[:, :, :, 7].rearrange("p (c h) b -> p c h b", c=8),
                        sc[:, :, :, 4].rearrange("p (c h) b -> p c h b", c=8))

            # ---------------- stage A + drain: 128 1-ch groups, striped
            pool_pb = [0]

            def next_way():
                w = pool_pb[0] % 3
                pool_pb[0] += 1
                return w

            claimed = {}
            with tc.tile_pool(name="psA", bufs=4, space="PSUM") as psA:
                for g in range(CL):
                    path, i = PLAN[g]
                    b, pos = divmod(i, 8)
                    pz = psA.tile([112, 2, 512], dt.float32)
                    nc.tensor.matmul(
                        pz[0:112, 0, 0:320],
                        aw_t[:, :, g, 0:112], xb_t[:, :, g, :],
                        start=True, stop=True, perf_mode=DR)
                    nc.tensor.matmul(
                        pz[0:84, 1, 0:320],
                        aw_t[:, :, g, 112:196], xb_t[:, :, g, :],
                        start=True, stop=True, perf_mode=DR)
                    v = pz[:, :, 0:320].rearrange("p c (b t) -> p c b t", t=T)
                    dest = ymax[:, _BASES[path] + 8 * b:_BASES[path] + 8 * b + 8]
                    if path in ("U1", "U4"):
                        ring = r4d if path == "U4" else None
                        if path == "U1":
                            if pos == 0:
                                claimed[("U1", b)] = next_way()
                            ring = p10[:, claimed[("U1", b)]]
                        stt_max(nc.vector, ring[:, 2 * pos:2 * pos + 2],
                                v[:, :, :, 0:10], v[:, :, :, 10:20])
                        if pos == 7:
                            if path == "U1":
                                pool_cascade(ring, dest)
                            else:
                                nc.vector.reduce_max(
                                    dest, r4d[:], axis=mybir.AxisListType.X)
                    else:  # U3h / U3p
                        way2 = (("U3h", b) if path == "U3h" else ("U3p", b))
                        if pos == 0:
                            claimed[way2] = len(claimed) % 2
                        w2 = s2d[:, claimed[way2]]
                        nc.scalar.copy(w2[:, 2 * pos:2 * pos + 2], v)
                        if pos == 7:
                            w = p10[:, next_way()]
                            eng = nc.vector if path == "U3h" else nc.gpsimd
                            stt_max(eng, w, w2[:, :, :, 0:10],
                                    w2[:, :, :, 10:20])
                            pool_cascade(w, dest)

            # ---------------- stage B + C per b-half
            with tc.tile_pool(name="psT", bufs=2, space="PSUM") as psT, \
                 tc.tile_pool(name="psC", bufs=2, space="PSUM") as psC:
                for bh in range(2):
                    for b in range(8 * bh, 8 * bh + 8):
                        ta = psT.tile([CL, 112], dt.float16, tag="ta")
                        nc.tensor.transpose(ta[:], ymax[:, :, 0, b], ident_t[:])
                        tb = psT.tile([CL, 84], dt.float16, tag="tb")
                        nc.tensor.transpose(tb[:], ymax[0:84, :, 1, b],
                                            ident_t[0:84, 0:84])
                        dsta = ymg[:, b, :, 1:5, 1:8]
                        srca = ta.rearrange("p (m h w) -> p m h w", m=4, w=7)
                        dstb = ymg[:, b, :, 5:8, 1:8]
                        srcb = tb.rearrange("p (m h w) -> p m h w", m=4, w=7)
                        if b % 2 == 0:
                            nc.vector.tensor_scalar_add(dsta, srca, 0.0)
                            nc.vector.tensor_scalar_add(dstb, srcb, 0.0)
                        else:
                            nc.scalar.copy(dsta, srca)
                            nc.scalar.copy(dstb, srcb)

                    for r in range(4):
                        pzc = psC.tile([CL, 8, HWQ], dt.float32)
                        for t9 in range(9):
                            kh, kw = divmod(t9, 3)
                            rhs = ymg[:, 8 * bh:8 * bh + 8, r,
                                      kh:kh + 7, kw:kw + 7]
                            nc.tensor.matmul(pzc[:], a2_t[:, 4 * t9 + r, :],
                                             rhs, start=(t9 == 0),
                                             stop=(t9 == 8))
                        zslice = zsb[:, r]
                        nc.gpsimd.scalar_tensor_tensor(
                            zslice, pzc[:], 1.0, zb_t[:, r], op0=op.mult,
                            op1=op.add)
                        nc.sync.dma_start(
                            out=zout_d[:, r, 8 * bh:8 * bh + 8, :], in_=zslice)

    nc.finalize()
    return nc


def _get_nc():
    global _NC_CACHE
    if _NC_CACHE is None:
        _NC_CACHE = build_bass()
    return _NC_CACHE


# ----------------------------------------------------------------- entry point
def kernel(x, dw_w, dw_b, conv_w, conv_b):
    global LAST_RESULTS
    from concourse.bass_utils import run_bass_kernel_spmd

    in_maps = [build_core_inputs(x, dw_w, dw_b, conv_w, conv_b, j)
               for j in range(NCORES)]
    nc = _get_nc()
    res = run_bass_kernel_spmd(nc, in_maps, core_ids=list(range(NCORES)),
                               trace=TRACE)
    LAST_RESULTS = res
    return assemble_output(res.results)


# revision 16
# speedup vs baseline: 1.2012x; 1.2012x over previous
"""Trainium2 Bass kernel for nn_DepthwiseConvOverTimeLayer.

Pipeline (per core, C-sharded 8 ways):
  stage A: depthwise 3x3 conv as per-channel banded matmul on PE,
           fp8e4m3 DoubleRow (K=49 split 25+24+pad), bias deferred.
           PSUM groups of 2 channels x (A:ho<4 112 rows | B:ho>=4 84 rows).
  drain:   temporal max over t=20 out of fp32 PSUM, split across
           DVE (pairwise max / direct reduce), ACT (copy to fp16 SBUF),
           Pool/GPSIMD (fp16 max cascades). Path per 4-group block is
           tunable via PATHS.
  stage B: PE transpose per b -> ymT [ch, b, m, 9x9-padded halo].
  stage C: conv2 as 36 block-diag matmuls (fp16, 9-tap PSUM accum),
           bias (dw_b + conv_b folded into host map zb) added on exit.

Channel coupling: conv2 group c2 consumes depthwise channel
c = 256*m2 + c2//4 at multiplier m = c2 % 4; core j owns c2 in
[128j, 128j+128) -> dw channels {256*(l//32) + 32j + (l%32)}.
"""

import numpy as np
import ml_dtypes

B, T, H, W, C, M = 16, 20, 7, 7, 1024, 4
KD = KP = 3
NCORES = 8
CL = 128          # dw channels per core
HWQ = 49
BT = B * T        # 320
F16 = np.float16
F8 = ml_dtypes.float8_e4m3fn

# Per-group (1 channel each) drain path stripe (period 16):
# V1: DVE pairwise max (t20->t10 into p10 ring) from PSUM.
# V3: ACT copy (t20 into s2 ring) + DVE TensorTensor halve -> p10.
# lvl2 per 8-group batch: TensorTensor max cascade on DVE (2x fp16) or Pool,
# alternating to balance load.
STRIPE16 = ["V1", "V3", "V1", "V3", "V1", "V3", "V1", "V3",
            "V1", "V3", "V3", "V1", "V3", "V1", "V3", "V3"]
_BASES = {"V1": 0, "V3": 56}


def _assignment():
    """Per-group (path, index-within-path); PERM[ymax col] = channel."""
    cnt = {p: 0 for p in _BASES}
    perm = np.zeros(CL, np.int64)
    plan = []
    for g in range(CL):
        path = STRIPE16[g % 16]
        i = cnt[path]
        cnt[path] += 1
        b, k = divmod(i, 8)
        perm[_BASES[path] + 8 * b + k] = g
        plan.append((path, i))
    return plan, perm


PLAN, PERM = _assignment()

TRACE = False
LAST_RESULTS = None
_NC_CACHE = None


# ----------------------------------------------------------------- host prep
def _core_channels(j):
    l = np.arange(CL)
    return 256 * (l // 32) + 32 * j + (l % 32)


def _col_of(m, ho, wo):
    """Stage-A output row index: A-half (ho<4) then B-half, m-major."""
    if ho < 4:
        return m * 28 + ho * 7 + wo
    return 112 + m * 21 + (ho - 4) * 7 + wo


def build_core_inputs(x, dw_w, dw_b, conv_w, conv_b, j):
    cj = _core_channels(j)

    # xb [25, 2, CL, BT] fp8: row hw_in = k*25+p, col bt = b*20+t
    xs = np.asarray(x, np.float32)[:, :, :, :, cj]           # [B,T,H,W,CL]
    xf = xs.transpose(2, 3, 4, 0, 1).reshape(HWQ, CL, BT)
    xb = np.zeros((50, CL, BT), np.float32)
    xb[:HWQ] = xf
    xb = xb.reshape(2, 25, CL, BT).transpose(1, 0, 2, 3)

    # aw [25, 2, CL, 196] fp8 banded depthwise lhsT (no bias row)
    wsel = np.asarray(dw_w, np.float32)[:, :, cj, :]         # [3,3,CL,4]
    aw = np.zeros((50, CL, 196), np.float32)
    for ho in range(H):
        for wo in range(W):
            for kh in range(KD):
                for kw in range(KD):
                    hi, wi = ho + kh - 1, wo + kw - 1
                    if 0 <= hi < H and 0 <= wi < W:
                        for m in range(M):
                            aw[hi * 7 + wi, :, _col_of(m, ho, wo)] = \
                                wsel[kh, kw, :, m]
    aw = aw.reshape(2, 25, CL, 196).transpose(1, 0, 2, 3)

    # a2 [CL, 36, CL] f16: conv2 block-diag weights.
    # a2[p_in=32*m2+s, t9*4+r, p_out=4*s+mo] = conv_w[c2=128j+4s+r, kh, kw, m2, mo]
    a2 = np.zeros((CL, 36, CL), np.float32)
    cw = np.asarray(conv_w, np.float32)
    s = np.arange(32)
    for t9 in range(9):
        kh, kw = divmod(t9, 3)
        for r in range(4):
            blk = cw[128 * j + 4 * s + r, kh, kw, :, :]       # [32, m2, mo]
            for m2 in range(4):
                a2[32 * m2 + s[:, None], t9 * 4 + r,
                   4 * s[:, None] + np.arange(4)[None, :]] = blk[:, m2, :]

    a2 = a2[PERM]

    # zb [CL, 4, 8, 49] f16: (dw_b + conv_b) folded through conv2, border-aware
    dwb_g = np.asarray(dw_b, np.float32).reshape(M, C)        # [m2, c2]
    c2s = 128 * j + np.arange(CL)
    # contrib[c2l, kh, kw, mo] = sum_m2 cw[c2, kh, kw, m2, mo] * dwb_g[m2, c2]
    contrib = np.einsum("ckhmn,mc->ckhn", cw[c2s], dwb_g[:, c2s])
    zmap = np.zeros((CL, M, HWQ), np.float32)                 # [c2l, mo, hw]
    cb = np.asarray(conv_b, np.float32)
    for ho in range(H):
        for wo in range(W):
            acc = cb[c2s].copy()                              # [c2l, mo]
            for kh in range(KD):
                for kw in range(KD):
                    if 0 <= ho + kh - 1 < H and 0 <= wo + kw - 1 < W:
                        acc += contrib[:, kh, kw]
            zmap[:, :, ho * 7 + wo] = acc
    # zb[po=4s+mo, r, hw] = zmap[c2l = 4s+r, mo, hw]
    zb = np.empty((CL, 4, HWQ), np.float32)
    for mo in range(4):
        for r in range(4):
            zb[4 * s + mo, r] = zmap[4 * s + r, mo]
    zb = np.repeat(zb[:, :, None, :], 8, axis=2)              # [CL, 4, 8, 49]

    ident = np.eye(112, dtype=np.float32)

    return {"xb": xb.astype(F8), "aw": aw.astype(F8),
            "a2": a2.astype(F16), "zb": zb.astype(F16),
            "ident": ident.astype(F16)}


def assemble_output(core_outs):
    """core_outs[j]['zout'] [CL=(4s+mo), 4=r, 16=b, 49=hw] -> (B,M,H,W,C)."""
    out = np.empty((B, M, H, W, C), np.float32)
    for j in range(NCORES):
        z = np.asarray(core_outs[j]["zout"]).astype(np.float32)
        z = z.reshape(32, 4, 4, B, HWQ)                 # s, mo, r, b, hw
        zz = z.transpose(3, 1, 4, 0, 2).reshape(B, M, H, W, CL)
        out[:, :, :, :, 128 * j:128 * j + 128] = zz
    return out


# ----------------------------------------------------------------- bass build
def build_bass():
    import concourse.mybir as mybir
    from concourse import bacc
    from concourse.tile import TileContext

    dt = mybir.dt
    op = mybir.AluOpType
    DR = mybir.MatmulPerfMode.DoubleRow
    nc = bacc.Bacc()

    xb_d = nc.dram_tensor("xb", [25, 2, CL, BT], dt.float8e4,
                          kind="ExternalInput")
    aw_d = nc.dram_tensor("aw", [25, 2, CL, 196], dt.float8e4,
                          kind="ExternalInput")
    a2_d = nc.dram_tensor("a2", [CL, 36, CL], dt.float16, kind="ExternalInput")
    zb_d = nc.dram_tensor("zb", [CL, 4, 8, HWQ], dt.float16,
                          kind="ExternalInput")
    ident_d = nc.dram_tensor("ident", [112, 112], dt.float16,
                             kind="ExternalInput")
    zout_d = nc.dram_tensor("zout", [CL, 4, B, HWQ], dt.float16,
                            kind="ExternalOutput")

    def stt_max(eng, out, in0, in1):
        eng.scalar_tensor_tensor(out, in0, 1.0, in1, op0=op.mult, op1=op.max)

    def tt_max(eng, out, in0, in1):
        # raw InstTensorTensor: has the 2x fp16 DVE fast path that
        # scalar_tensor_tensor lacks
        eng.add_instruction(mybir.InstTensorTensor(
            name=nc.get_next_instruction_name(),
            ins=[eng.lower_ap(in0), eng.lower_ap(in1)],
            outs=[eng.lower_ap(out)], op=op.max))

    with TileContext(nc) as tc:
        with tc.tile_pool(name="const", bufs=1) as cpool:
            xb_t = cpool.tile([25, 2, CL, BT], dt.float8e4)
            aw_t = cpool.tile([25, 2, CL, 196], dt.float8e4)
            a2_t = cpool.tile([CL, 36, CL], dt.float16)
            zb_t = cpool.tile([CL, 4, 8, HWQ], dt.float16)
            ident_t = cpool.tile([112, 112], dt.float16)
            ymax = cpool.tile([112, CL, 2, B], dt.float16)  # [row, ch, A|B, b]
            ymT = cpool.tile([CL, B, 4, 81], dt.float16)    # [ch, b, m, 9x9]
            zsb = cpool.tile([CL, 4, 8, HWQ], dt.float16)
            p10 = cpool.tile([112, 3, 16, B, 10], dt.float16)  # pool ring
            s2d = cpool.tile([112, 2, 16, B, T], dt.float16)   # ACT copy ring
            scd = cpool.tile([112, 16, B, 8], dt.float16)      # pool scratch
            scv = cpool.tile([112, 16, B, 8], dt.float16)      # DVE scratch

            nc.sync.dma_start(out=xb_t[:, :, 0:32, :], in_=xb_d[:, :, 0:32, :])
            nc.sync.dma_start(out=aw_t[:, :, 0:64, :], in_=aw_d[:, :, 0:64, :])
            nc.sync.dma_start(out=xb_t[:, :, 32:64, :],
                              in_=xb_d[:, :, 32:64, :])
            nc.sync.dma_start(out=xb_t[:, :, 64:96, :],
                              in_=xb_d[:, :, 64:96, :])
            nc.sync.dma_start(out=aw_t[:, :, 64:128, :],
                              in_=aw_d[:, :, 64:128, :])
            nc.sync.dma_start(out=xb_t[:, :, 96:128, :],
                              in_=xb_d[:, :, 96:128, :])
            nc.sync.dma_start(out=a2_t[:], in_=a2_d[:])
            nc.sync.dma_start(out=zb_t[:], in_=zb_d[:])
            nc.sync.dma_start(out=ident_t[:], in_=ident_d[:])

            # halo borders of ymT (interior is fully written in stage B)
            ymg = ymT.rearrange("p b m (hh ww) -> p b m hh ww", hh=9, ww=9)
            nc.gpsimd.memset(ymg[:, :, :, 0, :], 0.0)
            nc.gpsimd.memset(ymg[:, :, :, 8, :], 0.0)
            nc.gpsimd.memset(ymg[:, :, :, 1:8, 0], 0.0)
            nc.gpsimd.memset(ymg[:, :, :, 1:8, 8], 0.0)

            def cascade(eng, w, dest):
                """w [112,16,B,10] f16 -> max over t10 -> dest [112,8,2,B]."""
                sc = scd if eng is nc.gpsimd else scv
                mx = stt_max if eng is nc.gpsimd else tt_max
                mx(eng, sc[:, :, :, 0:5], w[:, :, :, 0:5], w[:, :, :, 5:10])
                mx(eng, sc[:, :, :, 5:7], sc[:, :, :, 0:2], sc[:, :, :, 2:4])
                mx(eng, sc[:, :, :, 7:8], sc[:, :, :, 5:6], sc[:, :, :, 6:7])
                mx(eng, dest,
                   sc[:, :, :, 7].rearrange("p (c h) b -> p c h b", c=8),
                   sc[:, :, :, 4].rearrange("p (c h) b -> p c h b", c=8))

            # ---------------- stage A + drain: 128 1-ch groups, striped
            pool_pb = [0]

            def next_way():
                w = pool_pb[0] % 3
                pool_pb[0] += 1
                return w

            claimed = {}
            casc_n = [0]

            def casc_engine():
                casc_n[0] += 1
                return nc.gpsimd if casc_n[0] % 2 == 0 else nc.vector

            with tc.tile_pool(name="psA", bufs=4, space="PSUM") as psA:
                for g in range(CL):
                    path, i = PLAN[g]
                    b, pos = divmod(i, 8)
                    pz = psA.tile([112, 2, 512], dt.float32)
                    nc.tensor.matmul(
                        pz[0:112, 0, 0:320],
                        aw_t[:, :, g, 0:112], xb_t[:, :, g, :],
                        start=True, stop=True, perf_mode=DR)
                    nc.tensor.matmul(
                        pz[0:84, 1, 0:320],
                        aw_t[:, :, g, 112:196], xb_t[:, :, g, :],
                        start=True, stop=True, perf_mode=DR)
                    v = pz[:, :, 0:320].rearrange("p c (b t) -> p c b t", t=T)
                    dest = ymax[:, _BASES[path] + 8 * b:_BASES[path] + 8 * b + 8]
                    if path == "V1":
                        if pos == 0:
                            claimed[("V1", b)] = next_way()
                        ring = p10[:, claimed[("V1", b)]]
                        stt_max(nc.vector, ring[:, 2 * pos:2 * pos + 2],
                                v[:, :, :, 0:10], v[:, :, :, 10:20])
                        if pos == 7:
                            cascade(casc_engine(), ring, dest)
                    else:  # V3
                        if pos == 0:
                            claimed[("V3", b)] = b % 2
                        w2 = s2d[:, claimed[("V3", b)]]
                        nc.scalar.copy(w2[:, 2 * pos:2 * pos + 2], v)
                        if pos == 7:
                            w = p10[:, next_way()]
                            tt_max(nc.vector, w, w2[:, :, :, 0:10],
                                   w2[:, :, :, 10:20])
                            cascade(casc_engine(), w, dest)

            # ---------------- stage B + C per b-half
            with tc.tile_pool(name="psT", bufs=2, space="PSUM") as psT, \
                 tc.tile_pool(name="psC", bufs=2, space="PSUM") as psC:
                for bh in range(2):
                    for b in range(8 * bh, 8 * bh + 8):
                        ta = psT.tile([CL, 112], dt.float16, tag="ta")
                        nc.tensor.transpose(ta[:], ymax[:, :, 0, b], ident_t[:])
                        tb = psT.tile([CL, 84], dt.float16, tag="tb")
                        nc.tensor.transpose(tb[:], ymax[0:84, :, 1, b],
                                            ident_t[0:84, 0:84])
                        dsta = ymg[:, b, :, 1:5, 1:8]
                        srca = ta.rearrange("p (m h w) -> p m h w", m=4, w=7)
                        dstb = ymg[:, b, :, 5:8, 1:8]
                        srcb = tb.rearrange("p (m h w) -> p m h w", m=4, w=7)
                        if b % 2 == 0:
                            nc.vector.tensor_scalar_add(dsta, srca, 0.0)
                            nc.vector.tensor_scalar_add(dstb, srcb, 0.0)
                        else:
                            nc.scalar.copy(dsta, srca)
                            nc.scalar.copy(dstb, srcb)

                    for r in range(4):
                        pzc = psC.tile([CL, 8, HWQ], dt.float32)
                        for t9 in range(9):
                            kh, kw = divmod(t9, 3)
                            rhs = ymg[:, 8 * bh:8 * bh + 8, r,
                                      kh:kh + 7, kw:kw + 7]
                            nc.tensor.matmul(pzc[:], a2_t[:, 4 * t9 + r, :],
                                             rhs, start=(t9 == 0),
                                             stop=(t9 == 8))
                        zslice = zsb[:, r]
                        nc.gpsimd.scalar_tensor_tensor(
                            zslice, pzc[:], 1.0, zb_t[:, r], op0=op.mult,
                            op1=op.add)
                        nc.sync.dma_start(
                            out=zout_d[:, r, 8 * bh:8 * bh + 8, :], in_=zslice)

    nc.finalize()
    return nc


def _get_nc():
    global _NC_CACHE
    if _NC_CACHE is None:
        _NC_CACHE = build_bass()
    return _NC_CACHE


# ----------------------------------------------------------------- entry point
def kernel(x, dw_w, dw_b, conv_w, conv_b):
    global LAST_RESULTS
    from concourse.bass_utils import run_bass_kernel_spmd

    in_maps = [build_core_inputs(x, dw_w, dw_b, conv_w, conv_b, j)
               for j in range(NCORES)]
    nc = _get_nc()
    res = run_bass_kernel_spmd(nc, in_maps, core_ids=list(range(NCORES)),
                               trace=TRACE)
    LAST_RESULTS = res
    return assemble_output(res.results)
